# revision 5
# baseline (speedup 1.0000x reference)
"""Trainium2 Bass kernel for nn_AttentionHead (causal single-head attention
with input projections), data-parallel over the batch dim on 8 NeuronCores.

Per-core computation (batch b):
  qh = q[b] @ Wq ; kh = k[b] @ Wk ; vh = v[b] @ Wv        [2048, 64]
  scores = (qh @ kh^T) * 8, causal-masked, softmax over s
  out[b] = softmax(scores) @ vh                            [2048, 64]

Implementation notes:
  - Host pre-transposes q/k/v to [E, L] layout (e on partitions) so the
    projection contraction dim lands on SBUF partitions.
  - Matmuls run in fp32r (TF32-like, RNE-11 mantissa rounding, 1 cyc/row at
    N>=256 vs 4 cyc/row for fp32).
  - Weight hi/lo split (W = hi + lo, both fp32r) rides along for free as
    extra lhsT columns: psum rows 0-63 = hi-part, 64-127 = lo-part, summed by
    one DVE add. Removes the W-side rounding error.
  - qh hi/lo split stacked on the contraction partitions of the QK matmul
    (lhsT = [qh_hi; qh_lo], rhs = [kh_r; kh_r]) removes the qh-side rounding
    error with zero extra matmul cycles.
  - Softmax: exact row max (DVE), exp fused with *8 scale and -8*max bias on
    ScalarE, output fp16.
  - P^T via PE transpose (fp16, exact); AV matmul in fp16 with a ones-column
    appended to vh so the softmax denominator Z falls out of the same
    accumulation (row 64 of the output), then out = out_unnorm / Z.
"""
import sys

if "/opt/trn_rl_repo" not in sys.path:
    sys.path.insert(0, "/opt/trn_rl_repo")

import numpy as np

N_CORES = 8
NB, L, S, E, D = 8, 2048, 2048, 1024, 64
P = 128
ECH = E // P          # 8 e-chunks
LCH = 4               # l/s chunks of 512 for projections
NLT = L // P          # 16 l-tiles
NST = S // P          # 16 s-tiles
CHUNK = 512

_PROGRAM = None


def _rne_f32r(a: np.ndarray) -> np.ndarray:
    """Round fp32 to fp32r (RNE, keep 11 mantissa bits) — matches TRN2 HW."""
    u = np.ascontiguousarray(a, dtype=np.float32).view(np.uint32).astype(np.uint64)
    lsb = (u >> np.uint64(12)) & np.uint64(1)
    r = ((u + np.uint64(0x7FF) + lsb) >> np.uint64(12)) << np.uint64(12)
    return r.astype(np.uint32).view(np.float32)


def _build_program():
    import concourse.bacc as bacc
    import concourse.mybir as mybir
    import concourse.tile as tile
    from concourse.bass import ds, ts

    F32 = mybir.dt.float32
    F32R = mybir.dt.float32r
    F16 = mybir.dt.float16
    Exp = mybir.ActivationFunctionType.Exp
    AX = mybir.AxisListType.X

    nc = bacc.Bacc(None, target_bir_lowering=False)

    qT = nc.declare_dram_parameter("qT", [P, ECH, L], F32R, isOutput=False)
    kT = nc.declare_dram_parameter("kT", [P, ECH, S], F32R, isOutput=False)
    vT = nc.declare_dram_parameter("vT", [P, ECH, S], F32R, isOutput=False)
    Wq_s = nc.declare_dram_parameter("Wq_s", [P, ECH, 2 * D], F32R, isOutput=False)
    Wk_s = nc.declare_dram_parameter("Wk_s", [P, ECH, 2 * D], F32R, isOutput=False)
    Wv_s = nc.declare_dram_parameter("Wv_s", [P, ECH, 2 * D], F32R, isOutput=False)
    id16_d = nc.declare_dram_parameter("id16", [P, P], F16, isOutput=False)
    id32_d = nc.declare_dram_parameter("id32", [P, P], F32, isOutput=False)
    dmask_d = nc.declare_dram_parameter("dmask", [P, P], F32, isOutput=False)
    out_d = nc.declare_dram_parameter("out", [L, D], F32, isOutput=True)

    with tile.TileContext(nc) as tc:
        with (
            tc.tile_pool(name="consts", bufs=1) as consts,
            tc.tile_pool(name="persist", bufs=1) as persist,
            tc.tile_pool(name="xstream", bufs=5) as xstream,
            tc.tile_pool(name="work", bufs=2) as work,
            tc.tile_pool(name="epool", bufs=2) as epool,
            tc.tile_pool(name="etpool", bufs=3) as etpool,
            tc.tile_pool(name="psA", bufs=2, space="PSUM") as psA,
            tc.tile_pool(name="psB", bufs=2, space="PSUM") as psB,
            tc.tile_pool(name="psC", bufs=2, space="PSUM") as psC,
            tc.tile_pool(name="psD", bufs=2, space="PSUM") as psD,
        ):
            # ---- constants ----
            wq_t = consts.tile([P, ECH, 2 * D], F32R, tag="wq")
            wk_t = consts.tile([P, ECH, 2 * D], F32R, tag="wk")
            wv_t = consts.tile([P, ECH, 2 * D], F32R, tag="wv")
            id16_t = consts.tile([P, P], F16, tag="id16")
            id32_t = consts.tile([P, P], F32, tag="id32")
            dmask_t = consts.tile([P, P], F32, tag="dmask")
            nc.sync.dma_start(out=wq_t, in_=Wq_s[:])
            nc.sync.dma_start(out=wk_t, in_=Wk_s[:])
            nc.sync.dma_start(out=wv_t, in_=Wv_s[:])
            nc.sync.dma_start(out=id16_t, in_=id16_d[:])
            nc.sync.dma_start(out=id32_t, in_=id32_d[:])
            nc.sync.dma_start(out=dmask_t, in_=dmask_d[:])

            # ---- persistent projected tensors ----
            # qsplit[i]: [128, 128] f32r; rows 0-63 = rnd(qh^T), 64-127 = lo
            qsplit = [persist.tile([P, P], F32R, tag=f"qsp{i}", name=f"qsp{i}") for i in range(NLT)]
            # kdup[c]: [128, 512] f32r; rows 0-63 = rnd(kh^T), 64-127 = copy
            kdup = [persist.tile([P, CHUNK], F32R, tag=f"kd{c}", name=f"kd{c}") for c in range(LCH)]
            # vones[j]: [128, 65] f16; cols 0-63 = vh rows, col 64 = 1.0
            vones = [persist.tile([P, D + 1], F16, tag=f"vo{j}", name=f"vo{j}") for j in range(NST)]
            for j in range(NST):
                nc.vector.memset(vones[j][:, D : D + 1], 1.0)

            def proj_chunk(x_dram, w_t, lc, kind):
                """Project 512 columns of x^T; route result per `kind`."""
                xt = xstream.tile([P, ECH, CHUNK], F32R, tag="xs")
                nc.sync.dma_start(out=xt, in_=x_dram[:, :, ds(lc * CHUNK, CHUNK)])
                ps = psA.tile([P, CHUNK], F32, tag="pj")
                for c in range(ECH):
                    nc.tensor.matmul(
                        ps, w_t[:, c, :], xt[:, c, :],
                        start=(c == 0), stop=(c == ECH - 1),
                    )
                lo_sb = work.tile([D, CHUNK], F32, tag="losb")
                nc.scalar.copy(out=lo_sb, in_=ps[D:, :])
                if kind == "q":
                    tmp = work.tile([D, CHUNK], F32, tag="qtmp")
                    nc.vector.tensor_add(out=tmp, in0=ps[:D, :], in1=lo_sb)
                    for j in range(4):
                        i = lc * 4 + j
                        sl = ds(j * P, P)
                        nc.vector.tensor_copy(out=qsplit[i][:D, :], in_=tmp[:, sl])
                        nc.vector.tensor_tensor(
                            out=qsplit[i][D:, :], in0=tmp[:, sl],
                            in1=qsplit[i][:D, :].bitcast(F32),
                            op=mybir.AluOpType.subtract,
                        )
                elif kind == "k":
                    kd = kdup[lc]
                    nc.vector.tensor_add(out=kd[:D, :], in0=ps[:D, :], in1=lo_sb)
                    nc.vector.tensor_copy(out=kd[D:, :], in_=kd[:D, :])
                else:  # v
                    vh16 = work.tile([D, CHUNK], F16, tag="vtmp")
                    nc.vector.tensor_add(out=vh16, in0=ps[:D, :], in1=lo_sb)
                    for j in range(4):
                        st = lc * 4 + j
                        pt = psC.tile([P, P], F16, tag="pt")
                        nc.tensor.transpose(
                            pt[:, :D], vh16[:, ds(j * P, P)], id16_t[:D, :D]
                        )
                        nc.vector.tensor_copy(out=vones[st][:, :D], in_=pt[:, :D])

            def attn_tile(i):
                ncols = (i + 1) * P
                nch = (ncols + CHUNK - 1) // CHUNK
                pscs = []
                for c2 in range(nch):
                    n = min(CHUNK, ncols - c2 * CHUNK)
                    psc = psB.tile([P, CHUNK], F32, tag="sc")
                    nc.tensor.matmul(
                        psc[:, :n], qsplit[i][:], kdup[c2][:, :n],
                        start=True, stop=True,
                    )
                    pscs.append((psc, n))
                # causal mask on the diagonal block
                dc, doff = i // 4, (i % 4) * P
                psc_d = pscs[dc][0]
                nc.vector.tensor_add(
                    out=psc_d[:, ds(doff, P)], in0=psc_d[:, ds(doff, P)],
                    in1=dmask_t,
                )
                ssb = work.tile([P, L], F32, tag="ssb")
                mx = work.tile([P, 4], F32, tag="mx")
                for c2, (psc, n) in enumerate(pscs):
                    nc.scalar.copy(out=ssb[:, ds(c2 * CHUNK, n)], in_=psc[:, :n])
                    nc.vector.reduce_max(
                        out=mx[:, c2 : c2 + 1], in_=ssb[:, ds(c2 * CHUNK, n)], axis=AX
                    )
                mxa = work.tile([P, 1], F32, tag="mxa")
                nc.vector.reduce_max(out=mxa, in_=mx[:, :nch], axis=AX)
                bias = work.tile([P, 1], F32, tag="bias")
                nc.scalar.mul(out=bias, in_=mxa, mul=-8.0)
                et_full = epool.tile([P, L], F16, tag="E")
                for c2, (psc, n) in enumerate(pscs):
                    nc.scalar.activation(
                        out=et_full[:, ds(c2 * CHUNK, n)],
                        in_=ssb[:, ds(c2 * CHUNK, n)],
                        func=Exp, bias=bias, scale=8.0,
                    )
                po = psD.tile([P, P], F32, tag="po")
                for j in range(i + 1):
                    pt2 = psC.tile([P, P], F16, tag="pt")
                    nc.tensor.transpose(pt2, et_full[:, ds(j * P, P)], id16_t)
                    et = etpool.tile([P, P], F16, tag="et")
                    nc.vector.tensor_copy(out=et, in_=pt2)
                    nc.tensor.matmul(
                        po[: D + 1, :], vones[j][:], et[:],
                        start=(j == 0), stop=(j == i),
                    )
                osb = work.tile([D + 1, P], F32, tag="osb")
                nc.vector.tensor_copy(out=osb, in_=po[: D + 1, :])
                po2 = psD.tile([P, P], F32, tag="po")
                nc.tensor.transpose(
                    po2[:, : D + 1], osb, id32_t[: D + 1, : D + 1]
                )
                zi = work.tile([P, 1], F32, tag="zi")
                nc.vector.reciprocal(zi, po2[:, D : D + 1])
                ob = work.tile([P, D], F32, tag="ob")
                nc.vector.tensor_scalar_mul(ob, po2[:, :D], zi)
                nc.sync.dma_start(out=out_d[ds(i * P, P), :], in_=ob)

            for lc in range(LCH):
                proj_chunk(kT, wk_t, lc, "k")
                proj_chunk(qT, wq_t, lc, "q")
                proj_chunk(vT, wv_t, lc, "v")
                for j in range(4):
                    attn_tile(lc * 4 + j)

    nc.finalize()
    return nc


def _get_program():
    global _PROGRAM
    if _PROGRAM is None:
        _PROGRAM = _build_program()
    return _PROGRAM


def make_in_maps(q, k, v, Wq, Wk, Wv):
    """Host-side sharding + layout prep. Returns one input map per core."""

    def w_stack(W):
        hi = _rne_f32r(W)
        lo = (W.astype(np.float32) - hi).astype(np.float32)
        ws = np.concatenate([hi, lo], axis=1)          # [E, 128]
        return np.ascontiguousarray(
            ws.reshape(ECH, P, 2 * D).transpose(1, 0, 2)
        )

    wq_s = w_stack(np.asarray(Wq, dtype=np.float32))
    wk_s = w_stack(np.asarray(Wk, dtype=np.float32))
    wv_s = w_stack(np.asarray(Wv, dtype=np.float32))
    id16 = np.eye(P, dtype=np.float16)
    id32 = np.eye(P, dtype=np.float32)
    dmask = np.where(
        np.arange(P)[None, :] > np.arange(P)[:, None], np.float32(-1e30), np.float32(0)
    ).astype(np.float32)

    in_maps = []
    for b in range(N_CORES):
        def xt(x):
            return np.ascontiguousarray(
                np.asarray(x, dtype=np.float32).T.reshape(ECH, P, -1).transpose(1, 0, 2)
            )

        in_maps.append({
            "qT": xt(q[b]), "kT": xt(k[b]), "vT": xt(v[b]),
            "Wq_s": wq_s, "Wk_s": wk_s, "Wv_s": wv_s,
            "id16": id16, "id32": id32, "dmask": dmask,
        })
    return in_maps


def kernel(q, k, v, Wq, Wk, Wv, attn_mask=None):
    from concourse.bass_utils import run_bass_kernel_spmd

    nc = _get_program()
    in_maps = make_in_maps(q, k, v, Wq, Wk, Wv)
    res = run_bass_kernel_spmd(nc, in_maps, core_ids=list(range(N_CORES)))
    out = np.stack([res.results[b]["out"] for b in range(N_CORES)], axis=0)
    return out.astype(np.float32)
